# revision 48
# baseline (speedup 1.0000x reference)
"""AttnBlock kernel for 8 Trainium2 NeuronCores.

Problem: x[4,512,64,64] f32 -> GroupNorm(2 groups, eps 1e-6) -> q,k,v 1x1 convs
-> attention over N=4096 positions with scale sqrt(512) (multiplied) -> proj
-> residual.

Sharding: 8 cores = 4 examples x 2 query-halves. Each core receives its
example's x with columns rotated so its half of the positions comes first
(softmax over keys is permutation invariant), computes GroupNorm + full k/v
and q for its 2048 positions, its 2048 attention rows, proj and residual.
No cross-core communication.

Design (evolved from a 3-pass hi/lo fp16 baseline at 1.32ms to ~0.35ms):
- fp16 single-pass matmuls everywhere (empirically rel_err ~8.9e-3 < 2e-2;
  logit noise ~0.3 rms only perturbs near-tie softmax rows).
- bk dropped (softmax shift-invariant); bv folded into bp host-side
  (softmax rows sum to 1), removing the v-bias matmul.
- x DMA'd once into a resident SBUF tile: shared by the GroupNorm stats
  pass (bn_stats + manual aggregation; bn_aggr is ~5.1us/op), the conv
  phase (GN affine fused into one ACT op via scale/bias APs), and the
  proj residual add.
- Per-chunk online softmax: each 512-key score chunk gets its own
  reduce_max + ACT exp (bias=-chunk max, accum_out=chunk sum) as soon as
  its 4 matmuls finish, so PSUM banks recycle continuously and the PE
  never waits on the max/exp chain; per-chunk alpha=exp(Mj-M) fixups on
  [128,8] tiles.
- Probability transposes moved off the PE onto the DMA xbar
  (dma_start_transpose -> [128, 32, 128] tiled P^T), which also removes
  the PSUM->SBUF evacuation copies from DVE.
- Softmax normalization (1/S) applied on ACT (scale AP) during PSUM
  evacuation of the attnV accumulator; attnV output transposed back via
  DMA xbar into [128, CT, 128] per block.
- Per-block proj (N=128 matmuls into one ping-ponged PSUM bank) keeps the
  pipeline tail short and avoids proj/score PSUM contention.
- Software pipeline per iteration: scores(nb) | attnV(nb-1) | proj(nb-2).
"""

import math

import numpy as np

import concourse.bacc as bacc
import concourse.mybir as mybir
import concourse.tile as tile
from concourse.bass_utils import run_bass_kernel_spmd

F32 = mybir.dt.float32
F16 = mybir.dt.float16

B, C, H, W = 4, 512, 64, 64
N = H * W            # 4096 key positions
NQ = N // 2          # 2048 query positions per core
P = 128              # partitions
CT = C // P          # 4 channel tiles
NCH = N // 512       # 8 key chunks of 512
NQB = NQ // P        # 16 query blocks of 128
MT = N // P          # 32 key tiles of 128
G = 2                # groupnorm groups
EPS = 1e-6
AX = mybir.AxisListType.X
ALU = mybir.AluOpType
ACTF = mybir.ActivationFunctionType

_CACHED_NC = None


def build_nc(loop_r: int = 1):
    nc = bacc.Bacc("TRN2", target_bir_lowering=False)

    x_d = nc.dram_tensor("x", [CT, P, N], F32, kind="ExternalInput")
    # packed, partition-major: one DMA each. [p, t, o] layouts.
    wqt_d = nc.dram_tensor("wqt", [P, CT, C], F16, kind="ExternalInput")  # scaled sqrt(C)
    wkt_d = nc.dram_tensor("wkt", [P, CT, C], F16, kind="ExternalInput")
    wvt_d = nc.dram_tensor("wvt", [P, CT, C], F16, kind="ExternalInput")
    wpt_d = nc.dram_tensor("wpt", [P, CT, C], F16, kind="ExternalInput")
    # per-channel params packed: [p, t, (bq, bp', gnw, gnb)]
    prm_d = nc.dram_tensor("prm", [P, CT, 4], F32, kind="ExternalInput")
    out_d = nc.dram_tensor("out", [CT, P, NQ], F32, kind="ExternalOutput")

    import contextlib

    with tile.TileContext(nc) as tc:
        loop_ctx = tc.For_i(0, loop_r, 1) if loop_r > 1 else contextlib.nullcontext()
        with (
            loop_ctx,
            tc.tile_pool(name="singles", bufs=1) as singles,
            tc.tile_pool(name="persist", bufs=1) as persist,
            tc.tile_pool(name="convw", bufs=1) as convw,
        ):
            ones_f32 = singles.tile([P, P], F32, name="ones_f32")
            nc.vector.memset(ones_f32, 1.0)
            inv256 = singles.tile([P, 1], F32, name="inv256")
            nc.vector.memset(inv256, 1.0 / (256.0 * 16.0))
            eps_t = singles.tile([P, 1], F32, name="eps_t")
            nc.vector.memset(eps_t, EPS)

            # weights and per-channel params (DMAs issued after the phase-1
            # x loads so they don't delay the stats critical path)
            wqt_all = convw.tile([P, CT, C], F16, name="wqt_all")
            wkt_all = convw.tile([P, CT, C], F16, name="wkt_all")
            wvt_all = convw.tile([P, CT, C], F16, name="wvt_all")
            wpt_all = persist.tile([P, CT, C], F16, name="wpt_all")
            prm = persist.tile([P, CT, 4], F32, name="prm")
            wqt = [wqt_all[:, t, :] for t in range(CT)]
            wkt = [wkt_all[:, t, :] for t in range(CT)]
            wvt = [wvt_all[:, t, :] for t in range(CT)]
            wpt = [wpt_all[:, t, :] for t in range(CT)]
            bq = [prm[:, t, 0:1] for t in range(CT)]
            bp = [prm[:, t, 1:2] for t in range(CT)]
            gnw = [prm[:, t, 2:3] for t in range(CT)]
            gnb = [prm[:, t, 3:4] for t in range(CT)]

            # persistent activations (all fp16 single precision)
            k_t = [persist.tile([P, N], F16, name=f"k{t}") for t in range(CT)]
            q_t = [persist.tile([P, NQ], F16, name=f"q{t}") for t in range(CT)]
            vT = [persist.tile([P, C], F16, name=f"vT{m}") for m in range(MT)]

            # ---------------- Phase 1: GroupNorm statistics ----------------
            # x is DMA'd ONCE into a resident SBUF tile shared by the stats
            # pass, the conv phase, and the proj residual.
            xall = persist.tile([P, CT, N], F32, name="xall")
            with (
                tc.tile_pool(name="stat_sb", bufs=1) as stat_sb,
                tc.tile_pool(name="stat_ps", bufs=2, space="PSUM") as stat_ps,
            ):
                stats6a = stat_sb.tile([P, CT, NCH, 6], F32, name="st6")
                stats6 = [stats6a[:, t] for t in range(CT)]
                # quarters 2,3 (cols 2048:4096) are never read by proj, so in
                # looped execution their loads can overlap the previous
                # iteration's attention phase; issue them first.
                for hf in (2, 3, 0, 1):
                    for t in range(CT):
                        xb = xall[:, t, hf * (N // 4):(hf + 1) * (N // 4)]
                        nc.sync.dma_start(
                            out=xb, in_=x_d[t][:, hf * (N // 4):(hf + 1) * (N // 4)])
                        for c2 in range(NCH // 4):
                            ch = hf * (NCH // 4) + c2
                            nc.vector.bn_stats(
                                out=stats6[t][:, ch, :], in_=xb[:, c2 * 512:(c2 + 1) * 512])
                # weights on the SAME (sync) queue so they are strictly
                # ordered after the x loads and can't steal HBM bandwidth
                # from the stats critical path
                nc.sync.dma_start(out=prm, in_=prm_d[:, :, :])
                nc.sync.dma_start(out=wkt_all, in_=wkt_d[:, :, :])
                nc.sync.dma_start(out=wqt_all, in_=wqt_d[:, :, :])
                nc.sync.dma_start(out=wvt_all, in_=wvt_d[:, :, :])
                nc.sync.dma_start(out=wpt_all, in_=wpt_d[:, :, :])
                # Manual aggregation (bn_aggr is ~5.1us/op): bn_stats gives
                # per-512-chunk even/odd sub-stats (count=256, mean, cnt*var).
                # stats2 cols: [sum16(mean)_t0..3 | sum16(var+mean^2)_t0..3];
                # the /16 folds into inv256 (= 1/(256*16)).
                stats2 = stat_sb.tile([P, 8], F32, name="stats2")
                m2s = stat_sb.tile([P, CT, 16], F32, name="m2s")
                sv = stats6a.rearrange("p t c (s f) -> p t (c s) f", f=3)
                means = sv[:, :, :, 1]
                ctv = sv[:, :, :, 2]
                nc.vector.tensor_tensor(out=m2s, in0=means, in1=means, op=ALU.mult)
                nc.vector.scalar_tensor_tensor(
                    out=m2s, in0=ctv, scalar=1.0 / 256.0, in1=m2s,
                    op0=ALU.mult, op1=ALU.add)
                nc.vector.reduce_sum(out=stats2[:, 0:4], in_=means, axis=AX)
                nc.vector.reduce_sum(out=stats2[:, 4:8], in_=m2s, axis=AX)
                # column sums / (256*16) -> [1, 8] on partition 0
                ps8 = stat_ps.tile([1, 8], F32, name="ps8")
                nc.tensor.matmul(ps8, inv256, stats2, start=True, stop=True)
                s8 = stat_sb.tile([1, 8], F32, name="s8")
                nc.vector.tensor_copy(s8, ps8)
                # per-group mean and E[x^2]: adjacent-pair sums
                gme = stat_sb.tile([1, 4], F32, name="gme")  # [mu_g0, mu_g1, e_g0, e_g1]
                s8v = s8.rearrange("p (f g two) -> p f g two", f=2, two=2)
                gmev = gme.rearrange("p (f g) -> p f g", f=2)
                nc.vector.tensor_tensor(
                    out=gmev[:, :, :], in0=s8v[:, :, :, 0], in1=s8v[:, :, :, 1], op=ALU.add)
                # broadcast to 128 partitions: [128, 4]
                psb = stat_ps.tile([P, 4], F32, name="psb")
                nc.tensor.matmul(psb, ones_f32[0:1, :], gme, start=True, stop=True)
                mu_e = stat_sb.tile([P, 4], F32, name="mu_e")
                nc.vector.tensor_copy(mu_e, psb)
                mu_bc = mu_e[:, 0:2]
                e_bc = mu_e[:, 2:4]
                var_bc = stat_sb.tile([P, 2], F32, name="var_bc")
                nc.vector.tensor_tensor(out=var_bc, in0=mu_bc, in1=mu_bc, op=ALU.mult)
                nc.vector.tensor_tensor(out=var_bc, in0=e_bc, in1=var_bc, op=ALU.subtract)
                sd = stat_sb.tile([P, 2], F32, name="sd")
                nc.scalar.activation(out=sd, in_=var_bc,
                                     func=ACTF.Sqrt, bias=eps_t, scale=1.0)
                rstd = stat_sb.tile([P, 2], F32, name="rstd")
                nc.vector.reciprocal(out=rstd, in_=sd)
                # per-channel-tile affine: h = a*x + b
                a_t = [persist.tile([P, 1], F32, name=f"a_t{t}") for t in range(CT)]
                b_t = [persist.tile([P, 1], F32, name=f"b_t{t}") for t in range(CT)]
                for t in range(CT):
                    g = t // 2
                    nc.vector.tensor_tensor(
                        out=a_t[t], in0=gnw[t], in1=rstd[:, g:g + 1], op=ALU.mult)
                    nc.vector.tensor_tensor(
                        out=b_t[t], in0=mu_bc[:, g:g + 1], in1=a_t[t], op=ALU.mult)
                    nc.vector.tensor_tensor(
                        out=b_t[t], in0=gnb[t], in1=b_t[t], op=ALU.subtract)

            # ---------------- Phase 2: h + q/k/v convs (streamed) ----------------
            with (
                tc.tile_pool(name="h16_pool", bufs=8) as h16_pool,
                tc.tile_pool(name="cq_ps", bufs=2, space="PSUM") as cq_ps,
                tc.tile_pool(name="ck_ps", bufs=2, space="PSUM") as ck_ps,
                tc.tile_pool(name="cv_ps", bufs=2, space="PSUM") as cv_ps,
            ):
                for ch in range(NCH):
                    sl = slice(ch * 512, (ch + 1) * 512)
                    h16 = []
                    for t in range(CT):
                        h16t = h16_pool.tile([P, 512], F16, name="h16", tag="h16")
                        # GN affine + fp16 cast fused on ACT, from resident x
                        nc.scalar.activation(
                            out=h16t, in_=xall[:, t, sl], func=ACTF.Identity,
                            bias=b_t[t], scale=a_t[t])
                        h16.append(h16t)
                    for o in range(CT):
                        osl = slice(o * P, (o + 1) * P)
                        kp = ck_ps.tile([P, 512], F32, name="kp", tag="kp")
                        for t in range(CT):
                            nc.tensor.matmul(
                                kp, wkt[t][:, osl], h16[t],
                                start=(t == 0), stop=(t == CT - 1))
                        # bk dropped (softmax shift-invariant): direct fp16 cast
                        # on DVE (ACT is the busier engine in this phase)
                        nc.vector.tensor_copy(k_t[o][:, sl], kp)
                        if ch < NCH // 2:
                            qp = cq_ps.tile([P, 512], F32, name="qp", tag="qp")
                            for t in range(CT):
                                nc.tensor.matmul(
                                    qp, wqt[t][:, osl], h16[t],
                                    start=(t == 0), stop=(t == CT - 1))
                            nc.scalar.activation(
                                out=q_t[o][:, sl], in_=qp, func=ACTF.Identity,
                                bias=bq[o])
                    # v conv, transposed output (bv folded into bp host-side)
                    for mb in range(4):
                        m = ch * 4 + mb
                        vp = cv_ps.tile([P, C], F32, name="vp", tag="vp")
                        for t in range(CT):
                            nc.tensor.matmul(
                                vp, h16[t][:, mb * P:(mb + 1) * P], wvt[t],
                                start=(t == 0), stop=(t == CT - 1))
                        nc.vector.tensor_copy(vT[m], vp)

            # xall stays resident through phase 3: proj reads the residual
            # directly from SBUF (no x re-read DMAs).

            # ---------------- Phase 3: attention ----------------
            with (
                tc.tile_pool(name="att_sb", bufs=1) as att_sb,
                tc.tile_pool(name="p_pool", bufs=2) as p_pool,
                tc.tile_pool(name="ptg_pool", bufs=2) as ptg_pool,
                tc.tile_pool(name="oc_pool", bufs=3) as oc_pool,
                tc.tile_pool(name="sc_ps", bufs=5, space="PSUM") as sc_ps,
                tc.tile_pool(name="o_ps", bufs=1, space="PSUM") as o_ps,
                tc.tile_pool(name="pp_ps", bufs=2, space="PSUM") as pp_ps,
                tc.tile_pool(name="fin_pool", bufs=2) as fin_pool,
            ):
                def emit_block(nb):
                    """Scores + per-chunk online softmax + DMA transposes."""
                    nsl = slice(nb * P, (nb + 1) * P)
                    nmx = att_sb.tile([P, 8], F32, name="nmx", tag="nmx", bufs=2)
                    sums = att_sb.tile([P, 8], F32, name="sums", tag="sums", bufs=2)
                    alph = att_sb.tile([P, 8], F32, name="alph", tag="alph", bufs=2)
                    sm = att_sb.tile([P, 4], F32, name="sm", tag="sm", bufs=2)
                    negM, s_tot, recip = (sm[:, i:i + 1] for i in range(3))
                    pt_b = p_pool.tile([P, N], F16, name="pexp", tag="pexp")
                    for ch in range(NCH):
                        msl = slice(ch * 512, (ch + 1) * 512)
                        sp = sc_ps.tile([P, 512], F32, name="sp", tag="sp")
                        for t in range(CT):
                            nc.tensor.matmul(
                                sp, q_t[t][:, nsl], k_t[t][:, msl],
                                start=(t == 0), stop=(t == CT - 1))
                        nc.vector.reduce_max(
                            out=nmx[:, ch:ch + 1], in_=sp, axis=AX, negate=True)
                        nc.scalar.activation(
                            out=pt_b[:, msl], in_=sp, func=ACTF.Exp,
                            bias=nmx[:, ch:ch + 1], scale=1.0,
                            accum_out=sums[:, ch:ch + 1])
                    # combine: negM = min_j(-Mj) = -M;  alpha_j = exp(Mj - M)
                    nc.vector.tensor_reduce(
                        out=negM, in_=nmx, axis=AX, op=ALU.min)
                    nc.scalar.activation(
                        out=alph, in_=nmx, func=ACTF.Exp, bias=negM, scale=-1.0)
                    # S = sum_j alpha_j * sums_j ; recip = 1/S
                    nc.vector.tensor_tensor(out=sums, in0=sums, in1=alph, op=ALU.mult)
                    nc.vector.reduce_sum(out=s_tot, in_=sums, axis=AX)
                    nc.vector.reciprocal(out=recip, in_=s_tot)
                    # rescale chunks by alpha_j, then DMA-transpose both halves
                    for ch in range(NCH):
                        msl = slice(ch * 512, (ch + 1) * 512)
                        nc.vector.tensor_scalar_mul(
                            out=pt_b[:, msl], in0=pt_b[:, msl],
                            scalar1=alph[:, ch:ch + 1])
                    ptg = ptg_pool.tile([P, MT, P], F16, name="ptg", tag="ptg")
                    for qu in range(4):
                        nc.sync.dma_start_transpose(
                            ptg[:, qu * (MT // 4):(qu + 1) * (MT // 4), :],
                            pt_b[:, qu * (N // 4):(qu + 1) * (N // 4)])
                    return (nb, ptg, recip)

                def emit_apply(st):
                    """attnV + normalize-on-ACT + DMA-transpose into oc."""
                    nb, ptg, recip = st
                    po = o_ps.tile([P, C], F32, name="po", tag="po")
                    for mt in range(MT):
                        nc.tensor.matmul(
                            po, ptg[:, mt, :], vT[mt],
                            start=(mt == 0), stop=(mt == MT - 1))
                    oT = fin_pool.tile([P, C], F16, name="oT", tag="oT", bufs=2)
                    nc.scalar.activation(
                        out=oT, in_=po, func=ACTF.Identity, scale=recip)
                    oc = oc_pool.tile([P, CT, P], F16, name="oc", tag="oc")
                    nc.sync.dma_start_transpose(oc, oT)
                    return oc

                def emit_projb(nb, oc):
                    """Per-block proj + bias + residual + store."""
                    nsl = slice(nb * P, (nb + 1) * P)
                    pp_b = pp_ps.tile([P, CT, P], F32, name="ppb", tag="ppb")
                    for o in range(CT):
                        for t in range(CT):
                            nc.tensor.matmul(
                                pp_b[:, o, :], wpt[t][:, o * P:(o + 1) * P],
                                oc[:, t, :], start=(t == 0), stop=(t == CT - 1))
                    fin = fin_pool.tile([P, CT, P], F32, name="fin", tag="fin", bufs=2)
                    for o in range(CT):
                        nc.scalar.activation(
                            out=fin[:, o, :], in_=pp_b[:, o, :],
                            func=ACTF.Identity, bias=bp[o])
                    nc.vector.tensor_tensor(
                        out=fin, in0=fin, in1=xall[:, :, nsl], op=ALU.add)
                    for o in range(CT):
                        nc.gpsimd.dma_start(out=out_d[o][:, nsl], in_=fin[:, o, :])

                # software pipeline: scores(nb) | attnV(nb-1) | proj(nb-2)
                prev = None
                applied = {}
                for it in range(NQB + 2):
                    st = emit_block(it) if it < NQB else None
                    if prev is not None:
                        applied[prev[0]] = emit_apply(prev)
                    pj = it - 2
                    if pj in applied:
                        emit_projb(pj, applied.pop(pj))
                    prev = st

    nc.compile()
    return nc


def _prep_shared(gn_w, gn_b, wq, bq, wk, bk, wv, bv, wp, bp):
    f32 = np.float32
    s = f32(math.sqrt(512.0))

    def pack(wT):  # [C, C] -> [P, CT, C] partition-major
        return np.ascontiguousarray(wT.reshape(CT, P, C).transpose(1, 0, 2))

    # bv folded into bp: out = Wp @ (attn + bv) + bp = Wp @ attn + (bp + Wp @ bv)
    bp_eff = bp.astype(f32) + wp.astype(f32) @ bv.astype(f32)
    prm = np.zeros((P, CT, 4), dtype=f32)
    prm[:, :, 0] = (bq.astype(f32) * s).reshape(CT, P).T
    prm[:, :, 1] = bp_eff.reshape(CT, P).T
    prm[:, :, 2] = gn_w.astype(f32).reshape(CT, P).T
    prm[:, :, 3] = gn_b.astype(f32).reshape(CT, P).T
    shared = {
        "wqt": pack((wq.T * s).astype(f32)).astype(np.float16),
        "wkt": pack(wk.T.astype(f32)).astype(np.float16),
        "wvt": pack(wv.T.astype(f32)).astype(np.float16),
        "wpt": pack(wp.T.astype(f32)).astype(np.float16),
        "prm": prm,
    }
    return shared


def _make_in_maps(inputs):
    x = np.asarray(inputs["x"], dtype=np.float32)
    args = [np.asarray(inputs[k], dtype=np.float32) for k in
            ("gn_w", "gn_b", "wq", "bq", "wk", "bk", "wv", "bv", "wp", "bp")]
    shared = _prep_shared(*args)
    in_maps = []
    for core in range(8):
        b, half = core // 2, core % 2
        xb = x[b].reshape(C, N)
        if half:
            xb = np.concatenate([xb[:, NQ:], xb[:, :NQ]], axis=1)
        m = dict(shared)
        m["x"] = np.ascontiguousarray(xb.reshape(CT, P, N))
        in_maps.append(m)
    return in_maps


def kernel(x, gn_w, gn_b, wq, bq, wk, bk, wv, bv, wp, bp):
    global _CACHED_NC
    if _CACHED_NC is None:
        _CACHED_NC = build_nc()
    nc = _CACHED_NC

    in_maps = _make_in_maps(dict(x=x, gn_w=gn_w, gn_b=gn_b, wq=wq, bq=bq, wk=wk,
                                 bk=bk, wv=wv, bv=bv, wp=wp, bp=bp))
    res = run_bass_kernel_spmd(nc, in_maps, core_ids=list(range(8)))

    y = np.empty((B, C, N), dtype=np.float32)
    for core in range(8):
        b, half = core // 2, core % 2
        y[b][:, half * NQ:(half + 1) * NQ] = res.results[core]["out"].reshape(C, NQ)
    return y.reshape(B, C, H, W)
